# revision 3
# baseline (speedup 1.0000x reference)
"""Causal attention (single head) on 8 Trainium2 NeuronCores.

Problem: x[4096,1024], Wq/Wk/Wv[1024,1024] (torch Linear layout, applied as
x @ W.T); out = renormalized-causal-softmax(Q K^T / 32) @ V, fp32, [4096,1024].

Distribution (hardcoded for S=4096, D=1024, 8 cores):
  - Q rows are sharded STRIDED: core c owns rows c::8.  With 128-row q-tiles,
    tile j of every core spans global rows [1024j, 1024j+1024), so every core
    has identical causal trip counts (SPMD: one program, data-only variation).
    The intra-tile causal mask depends only on (c + 8i >= kk) and is shipped
    as a per-core input tensor.
  - K/V rows are sharded CONTIGUOUS: core c computes K,V for rows
    [512c, 512c+512) in bf16; shards are exchanged with AllGather collectives.
    The CC stream is the scarce resource (~40us per 8MB-output AllGather,
    strictly serial), so the phase order maximizes compute under it:
    K^T projection first -> AG(KT); V projection -> AG(V dh=0), AG(V dh=1);
    Q^T projection + S/exp/transpose attention work run under the AGs; the
    P@V matmuls for each d-half are gated only on their own AG.
  - All matmuls run in bf16 (fp32 matmul is 4x slower on the PE); PSUM
    accumulation is fp32.  Softmax: scores/32 are within +-3 at this problem
    scale so exp needs no max-subtraction; the causal mask is an additive
    -30000 on pre-exp scores (data, not code).  The reference's
    "softmax -> tril -> renormalize" is algebraically identical to
    masked exp / masked sum (the dense softmax denominator cancels).
"""

import numpy as np
import ml_dtypes

S, D, NC_N = 4096, 1024, 8
QROWS = S // NC_N            # 512 q rows per core
KVROWS = S // NC_N           # 512 kv rows per core
NQT = QROWS // 128           # 4 q-tiles of 128 rows per core
DC = D // 128                # 8 contraction chunks
BF16 = ml_dtypes.bfloat16

_CACHE = {}


def _build():
    import concourse.bass as bass
    import concourse.mybir as mybir
    import concourse.tile as tile
    from concourse import bacc
    from concourse.masks import make_identity

    fp32 = mybir.dt.float32
    bf16 = mybir.dt.bfloat16
    AX = mybir.AxisListType.X

    nc = bacc.Bacc("TRN2", target_bir_lowering=False, debug=False,
                   num_devices=NC_N, enable_asserts=False)

    xt_q = nc.dram_tensor("xt_q", [D, QROWS], bf16, kind="ExternalInput").ap()
    xt_kv = nc.dram_tensor("xt_kv", [D, KVROWS], bf16, kind="ExternalInput").ap()
    wqt = nc.dram_tensor("wqt", [D, D], bf16, kind="ExternalInput").ap()
    wkt = nc.dram_tensor("wkt", [D, D], bf16, kind="ExternalInput").ap()
    wvt = nc.dram_tensor("wvt", [D, D], bf16, kind="ExternalInput").ap()
    maskneg = nc.dram_tensor("maskneg", [128, 1024], fp32, kind="ExternalInput").ap()
    out = nc.dram_tensor("out", [QROWS, D], fp32, kind="ExternalOutput").ap()

    rg = [list(range(NC_N))]
    inv_sqrt_d = 1.0 / np.sqrt(np.float32(D))

    with tile.TileContext(nc) as tc:
        with (
            tc.tile_pool(name="dram", bufs=1, space="DRAM") as dram,
            tc.tile_pool(name="const", bufs=1) as cpool,
            tc.tile_pool(name="kvres", bufs=1) as kvpool,
            tc.tile_pool(name="stats", bufs=2) as stpool,
        ):
            kt_cc_in = dram.tile([D, KVROWS], bf16, name="kt_cc_in")
            v_cc_in0 = dram.tile([KVROWS, 512], bf16, name="v_cc_in0")
            v_cc_in1 = dram.tile([KVROWS, 512], bf16, name="v_cc_in1")
            kt_cc_out = dram.tile([NC_N, D, KVROWS], bf16, name="kt_cc_out",
                                  addr_space="Shared")
            v_cc_out0 = dram.tile([NC_N, KVROWS, 512], bf16, name="v_cc_out0",
                                  addr_space="Shared")
            v_cc_out1 = dram.tile([NC_N, KVROWS, 512], bf16, name="v_cc_out1",
                                  addr_space="Shared")

            ident = cpool.tile([128, 128], bf16, name="ident")
            make_identity(nc, ident)
            mask_sb = cpool.tile([128, 1024], fp32, name="mask_sb")
            nc.scalar.dma_start(mask_sb[:], maskneg[:])

            # gathered K^T: ktf[r][p, dc*512+j] = K[512r+j, 128dc+p]
            ktf = [kvpool.tile([128, DC * 512], bf16, name=f"ktf{r}")
                   for r in range(NC_N)]
            # gathered V d-halves: vf[dh][r][p, sl*512+j] = V[512r+128sl+p, 512dh+j]
            vf = [[kvpool.tile([128, 4 * 512], bf16, name=f"vf{dh}_{r}")
                   for r in range(NC_N)] for dh in range(2)]
            # Q^T resident: qt_sb[p, dc*512 + 128qt + i] = Q[q=128qt+i, 128dc+p]
            qt_sb = kvpool.tile([128, DC * 512], bf16, name="qt_sb")

            # ---------------- phase 1: projections + gathers ----------------
            with (
                tc.tile_pool(name="wpool", bufs=12) as wpool,
                tc.tile_pool(name="xpool", bufs=1) as xpool,
                tc.tile_pool(name="loc", bufs=4) as locpool,
                tc.tile_pool(name="ppsum", bufs=3, space="PSUM") as ppsum,
            ):
                # K-projection inputs first (critical path to the CC stream)
                wk, xkv = [], []
                for dc in range(DC):
                    tw = wpool.tile([128, D], bf16, name=f"wk{dc}", tag="w")
                    nc.scalar.dma_start(tw[:], wkt[dc * 128:(dc + 1) * 128, :])
                    wk.append(tw)
                    tx = xpool.tile([128, KVROWS], bf16, name=f"xkv{dc}")
                    nc.scalar.dma_start(tx[:], xt_kv[dc * 128:(dc + 1) * 128, :])
                    xkv.append(tx)

                # K^T_local[d, s] = (Wk @ x_kv^T): lhsT = Wk^T chunk, rhs = x_kv^T
                for po in range(DC):
                    ps = ppsum.tile([128, 512], fp32, tag="pp")
                    for dc in range(DC):
                        nc.tensor.matmul(ps[:], wk[dc][:, po * 128:(po + 1) * 128],
                                         xkv[dc][:],
                                         start=(dc == 0), stop=(dc == DC - 1))
                    loc = locpool.tile([128, 512], bf16, tag="loc")
                    nc.vector.tensor_copy(loc[:], ps[:])
                    nc.sync.dma_start(kt_cc_in[po * 128:(po + 1) * 128, :], loc[:])

                nc.gpsimd.collective_compute(
                    "AllGather", mybir.AluOpType.bypass, replica_groups=rg,
                    ins=[kt_cc_in[:]], outs=[kt_cc_out[:]])

                # V_local[s, d] = x_kv @ Wv^T: lhsT = x_kv^T chunk, rhs = Wv^T
                wv = []
                for dc in range(DC):
                    tw = wpool.tile([128, D], bf16, name=f"wv{dc}", tag="w")
                    nc.scalar.dma_start(tw[:], wvt[dc * 128:(dc + 1) * 128, :])
                    wv.append(tw)
                for dh in range(2):
                    v_cc_in = v_cc_in0 if dh == 0 else v_cc_in1
                    for st in range(4):
                        ps = ppsum.tile([128, 512], fp32, tag="pp")
                        for dc in range(DC):
                            nc.tensor.matmul(
                                ps[:], xkv[dc][:, st * 128:(st + 1) * 128],
                                wv[dc][:, dh * 512:(dh + 1) * 512],
                                start=(dc == 0), stop=(dc == DC - 1))
                        loc = locpool.tile([128, 512], bf16, tag="loc")
                        nc.vector.tensor_copy(loc[:], ps[:])
                        nc.sync.dma_start(v_cc_in[st * 128:(st + 1) * 128, :],
                                          loc[:])
                    nc.gpsimd.collective_compute(
                        "AllGather", mybir.AluOpType.bypass, replica_groups=rg,
                        ins=[v_cc_in[:]],
                        outs=[(v_cc_out0 if dh == 0 else v_cc_out1)[:]])

                # Q^T[d, q]: lhsT = Wq^T chunk, rhs = x_q^T  -> straight to SBUF
                wq, xq = [], []
                for dc in range(DC):
                    tw = wpool.tile([128, D], bf16, name=f"wq{dc}", tag="w")
                    nc.scalar.dma_start(tw[:], wqt[dc * 128:(dc + 1) * 128, :])
                    wq.append(tw)
                    tx = xpool.tile([128, QROWS], bf16, name=f"xq{dc}")
                    nc.scalar.dma_start(tx[:], xt_q[dc * 128:(dc + 1) * 128, :])
                    xq.append(tx)
                for po in range(DC):
                    ps = ppsum.tile([128, 512], fp32, tag="pp")
                    for dc in range(DC):
                        nc.tensor.matmul(ps[:], wq[dc][:, po * 128:(po + 1) * 128],
                                         xq[dc][:],
                                         start=(dc == 0), stop=(dc == DC - 1))
                    nc.vector.tensor_copy(qt_sb[:, po * 512:(po + 1) * 512], ps[:])

            # ---------------- phase 2: pull gathered K/V into SBUF ----------
            for r in range(NC_N):
                nc.sync.dma_start(
                    ktf[r][:].rearrange("p (a j) -> p a j", a=DC),
                    kt_cc_out[r].rearrange("(a p) j -> p a j", p=128))
            for r in range(NC_N):
                nc.scalar.dma_start(
                    vf[0][r][:].rearrange("p (a j) -> p a j", a=4),
                    v_cc_out0[r].rearrange("(a p) j -> p a j", p=128))
            for r in range(NC_N):
                nc.scalar.dma_start(
                    vf[1][r][:].rearrange("p (a j) -> p a j", a=4),
                    v_cc_out1[r].rearrange("(a p) j -> p a j", p=128))

            # ---------------- phase 3: attention ----------------
            with (
                tc.tile_pool(name="pbuf", bufs=2) as ppool,
                tc.tile_pool(name="ptbuf", bufs=2) as ptpool,
                tc.tile_pool(name="obuf", bufs=2) as opool,
                tc.tile_pool(name="spsum", bufs=2, space="PSUM") as spsum,
                tc.tile_pool(name="tpsum", bufs=2, space="PSUM") as tpsum,
                tc.tile_pool(name="opsum", bufs=2, space="PSUM") as opsum,
            ):
                for qt in range(NQT):
                    nkb = 2 * (qt + 1)          # 512-wide key blocks
                    nkc = 4 * nkb               # 128-wide key chunks
                    p_sb = ppool.tile([128, S], bf16, tag="p")
                    pt_sb = ptpool.tile([128, S], bf16, tag="pt")
                    dpart = stpool.tile([128, 8], fp32, tag="dpart")
                    den = stpool.tile([128, 1], fp32, tag="den")
                    recip = stpool.tile([128, 1], fp32, tag="recip")

                    for kb in range(nkb):
                        ps = spsum.tile([128, 512], fp32, tag="s")
                        for dc in range(DC):
                            nc.tensor.matmul(
                                ps[:],
                                qt_sb[:, dc * 512 + qt * 128: dc * 512 + (qt + 1) * 128],
                                ktf[kb][:, dc * 512:(dc + 1) * 512],
                                start=(dc == 0), stop=(dc == DC - 1))
                        if kb >= nkb - 2:
                            moff = (kb - (nkb - 2)) * 512
                            nc.vector.tensor_add(
                                ps[:], ps[:], mask_sb[:, moff:moff + 512])
                        nc.scalar.activation(
                            p_sb[:, kb * 512:(kb + 1) * 512], ps[:],
                            mybir.ActivationFunctionType.Exp,
                            bias=0.0, scale=float(inv_sqrt_d),
                            accum_out=dpart[:, kb:kb + 1])

                    nc.vector.reduce_sum(den[:], dpart[:, 0:nkb], axis=AX)
                    nc.vector.reciprocal(recip[:], den[:])

                    for kc in range(nkc):
                        pst = tpsum.tile([128, 128], bf16, tag="t")
                        nc.tensor.transpose(
                            pst[:], p_sb[:, kc * 128:(kc + 1) * 128], ident[:])
                        nc.vector.tensor_copy(pt_sb[:, kc * 128:(kc + 1) * 128],
                                              pst[:])

                    o_sb = opool.tile([128, D], fp32, tag="o")
                    for dh in range(2):
                        pso = opsum.tile([128, 512], fp32, tag="po")
                        for kc in range(nkc):
                            r, sl = kc // 4, kc % 4
                            nc.tensor.matmul(
                                pso[:], pt_sb[:, kc * 128:(kc + 1) * 128],
                                vf[dh][r][:, sl * 512:(sl + 1) * 512],
                                start=(kc == 0), stop=(kc == nkc - 1))
                        nc.vector.tensor_scalar_mul(
                            o_sb[:, dh * 512:(dh + 1) * 512], pso[:], recip[:])
                    nc.sync.dma_start(out[qt * 128:(qt + 1) * 128, :], o_sb[:])

    nc.compile()
    return nc


def _get_nc():
    if "nc" not in _CACHE:
        _CACHE["nc"] = _build()
    return _CACHE["nc"]


def make_in_maps(x, Wq, Wk, Wv):
    x_bf = np.ascontiguousarray(x).astype(BF16)
    wqt = np.ascontiguousarray(Wq.astype(BF16).T)
    wkt = np.ascontiguousarray(Wk.astype(BF16).T)
    wvt = np.ascontiguousarray(Wv.astype(BF16).T)
    in_maps = []
    for c in range(NC_N):
        xt_q = np.ascontiguousarray(x_bf[c::NC_N].T)
        xt_kv = np.ascontiguousarray(x_bf[c * KVROWS:(c + 1) * KVROWS].T)
        i = np.arange(128)[:, None]
        kk = np.arange(1024)[None, :]
        maskneg = np.where(c + 8 * i >= kk, 0.0, -30000.0).astype(np.float32)
        in_maps.append({"xt_q": xt_q, "xt_kv": xt_kv, "wqt": wqt,
                        "wkt": wkt, "wvt": wvt, "maskneg": maskneg})
    return in_maps


def run(in_maps, trace=False, tmpdir=None, trace_cores=None):
    from concourse.bass_utils import run_bass_kernel_spmd
    nc = _get_nc()
    return run_bass_kernel_spmd(nc, in_maps, core_ids=list(range(NC_N)),
                                trace=trace, tmpdir=tmpdir,
                                trace_cores=trace_cores)


def kernel(x, Wq, Wk, Wv):
    res = run(make_in_maps(np.asarray(x), np.asarray(Wq),
                           np.asarray(Wk), np.asarray(Wv)))
    full = np.empty((S, D), np.float32)
    for c in range(NC_N):
        full[c::NC_N] = res.results[c]["out"]
    return full
